# revision 74
# baseline (speedup 1.0000x reference)
"""Trainium2 Bass kernel: multi-head attention (B=2, S=2048, E=1024, H=16).

Sharding: 8 cores = 2 batches x 4 head-groups. Core c handles batch c//4 and
heads [4*(c%4), 4*(c%4)+4) (256 feature columns of the projections).

Per-core device program (fp16 operands, fp32 psum accumulation):
  - inputs: xT [E,S] (host-transposed x[b]), wqT/wkT/wvT [E,256] (host-
    transposed row-slices of Wq/Wk/Wv), woT [256,E] (host-transposed column
    slice of Wo), ident [128,128] identity for PE transposes.
  - qT,kT [256,S] = (x @ W^T)^T per head-group in [f,s] layout; v [S,256] in
    [s,f] layout with a ones column appended per head (softmax denominator).
  - per (head-pair, qi-chunk): scores^T tiles [128 kj, 512 qi] on PE, exp on
    ACT (sm_scale folded into the activation scale) to fp16; attn@v in the
    flipped orientation out[q, d] = et[k, q]^T @ v[k, d+1] streaming only 65
    rows per matmul (half the PE cost of streaming queries), accumulated over
    the 16 k-tiles; col 64 of the accumulator is the softmax denominator.
  - normalize fused into psum evacuation: per-partition reciprocal multiply
    (tensor_scalar with [P,1] scalar AP) -> o [s, f] fp16; PE transpose with
    the identity -> o^T [f, s] for the output projection.
  - out_partial [S,E] fp16 = o^T^T @ Wo^T column-slice; host sums 4 partials
    per batch in fp32 and adds bo.

Schedule: emission order = Tile scheduler priority, and in the Tile
framework the emission order also defines dependency direction, so every
producer must be emitted before its first consumer. All psum-chain
generators (projections, output projections) flow FIFO through one labeled
filler deque; ensure(label) pumps the queue until a generator is fully
emitted, invoked right before its first consumer's emission point. Chunk-0
k/q projections are emitted inline (interleaved in et-halves against the
staggered x DMA); everything else is pumped into the PE idle gaps of the
ACT-bound attention inner loop. The Exp table is preloaded and dummy
matmuls pre-ramp the PE p-state during the initial DMA wait. The last
chunk's normalize/transpose/output-projection is a per-q-tile pipeline
with evacuation split across ACT and DVE to keep the post-last-exp tail
short.
"""

from collections import deque

import numpy as np

import concourse.tile as tile
import concourse.mybir as mybir
from concourse import bacc
from concourse.bass_utils import run_bass_kernel_spmd

B, S, E, H, D = 2, 2048, 1024, 16, 64
NCORES = 8
GPB = NCORES // B      # head-groups (cores) per batch = 4
HPC = H // GPB         # heads per core = 4
FPC = HPC * D          # feature cols per core = 256
SM = float(D) ** -0.5  # softmax scale

F32 = mybir.dt.float32
F16 = mybir.dt.float16

P = 128
NE = E // P            # 8 e-tiles
NST = S // P           # 16 s-tiles (key tiles)
NQ = 4                 # qi chunks
QC = S // NQ           # 512
NQT = QC // P          # 4 q-tiles per chunk
FT = FPC // P          # 2 f-tiles per core


def _build():
    nc = bacc.Bacc("TRN2", target_bir_lowering=False, debug=False)

    xT_d = nc.dram_tensor("xT", [E, S], F16, kind="ExternalInput")
    wq_d = nc.dram_tensor("wqT", [E, FPC], F16, kind="ExternalInput")
    wk_d = nc.dram_tensor("wkT", [E, FPC], F16, kind="ExternalInput")
    wv_d = nc.dram_tensor("wvT", [E, FPC], F16, kind="ExternalInput")
    wo_d = nc.dram_tensor("woT", [FPC, E], F16, kind="ExternalInput")
    id_d = nc.dram_tensor("ident", [P, P], F16, kind="ExternalInput")
    out_d = nc.dram_tensor("out", [S, E], F16, kind="ExternalOutput")

    with tile.TileContext(nc) as tc:
        with (
            tc.tile_pool(name="wpool", bufs=1) as wpool,
            tc.tile_pool(name="xpool", bufs=1) as xpool,
            tc.tile_pool(name="qkpool", bufs=1) as qkpool,
            tc.tile_pool(name="vpool", bufs=1) as vpool,
            tc.tile_pool(name="opool", bufs=1) as opool,
            tc.tile_pool(name="epool", bufs=10) as epool,
            tc.tile_pool(name="spool", bufs=3) as spool,
            tc.tile_pool(name="outpool", bufs=4) as outpool,
            tc.tile_pool(name="pspool", bufs=2, space="PSUM") as pspool,
            tc.tile_pool(name="popool", bufs=2, space="PSUM") as popool,
            tc.tile_pool(name="oaccpool", bufs=2, space="PSUM") as oaccpool,
        ):
            # ---- weights / constants -------------------------------------
            wq = wpool.tile([P, NE, FPC], F16, name="wq")
            wk = wpool.tile([P, NE, FPC], F16, name="wk")
            wv = wpool.tile([P, NE, FPC], F16, name="wv")
            wo = wpool.tile([P, FT, E], F16, name="wo")
            ident = wpool.tile([P, P], F16, name="ident")
            # DMA issue is spread across engine sequencers (SP/DVE/ACT/Pool)
            # so the per-issue ~650ns sequencer cost doesn't serialize the
            # startup transfers; the DMA bus itself drains them in order.
            xT_r = xT_d.ap().rearrange("(t p) s -> p t s", p=P)
            xts = xpool.tile([P, NE, S], F16, name="xts")
            nc.sync.dma_start(out=wk, in_=wk_d.ap().rearrange("(t p) f -> p t f", p=P))
            nc.scalar.dma_start(out=xts[:, 0:4, 0:QC], in_=xT_r[:, 0:4, 0:QC])
            nc.scalar.dma_start(out=wq, in_=wq_d.ap().rearrange("(t p) f -> p t f", p=P))
            nc.sync.dma_start(out=xts[:, 4:8, 0:QC], in_=xT_r[:, 4:8, 0:QC])
            nc.sync.dma_start(out=wv, in_=wv_d.ap().rearrange("(t p) f -> p t f", p=P))
            nc.sync.dma_start(out=xts[:, :, QC : 2 * QC], in_=xT_r[:, :, QC : 2 * QC])
            nc.sync.dma_start(
                out=xts[:, :, 2 * QC : 3 * QC], in_=xT_r[:, :, 2 * QC : 3 * QC]
            )
            nc.sync.dma_start(out=ident, in_=id_d.ap())
            nc.sync.dma_start(
                out=xts[:, :, 3 * QC : 4 * QC], in_=xT_r[:, :, 3 * QC : 4 * QC]
            )
            nc.sync.dma_start(out=wo, in_=wo_d.ap().rearrange("(t p) g -> p t g", p=P))

            # Warmups AFTER the DMA issues so they don't delay transfers:
            # preload the Exp table (saves the ~1.3us load on the first real
            # exp) and pre-ramp the PE p-state with dummy matmuls during the
            # initial DMA wait so projections start at full clock.
            warm = wpool.tile([1, 1], F32, name="warm")
            nc.vector.memset(warm, 0.0)
            warm2 = wpool.tile([1, 1], F32, name="warm2")
            nc.scalar.activation(
                out=warm2, in_=warm, func=mybir.ActivationFunctionType.Exp
            )
            wl = wpool.tile([1, 1], F16, name="wl")
            nc.vector.memset(wl, 0.0)
            wr = wpool.tile([1, QC], F16, name="wr")
            nc.vector.memset(wr, 0.0)
            for i in range(10):
                pw = pspool.tile([1, QC], F32, name="pw", tag="ps_s")
                nc.tensor.matmul(pw, wl, wr, start=True, stop=True)

            kts = [qkpool.tile([P, S], F16, name=f"kt{ft}", tag=f"kt{ft}") for ft in range(FT)]
            qts = [qkpool.tile([P, S], F16, name=f"qt{ft}", tag=f"qt{ft}") for ft in range(FT)]
            ots = [opool.tile([P, S], F16, name=f"ot{ft}", tag=f"ot{ft}") for ft in range(FT)]
            v_tiles = [
                vpool.tile([P, HPC, D + 1], F16, name=f"v{st}", tag=f"v{st}")
                for st in range(NST)
            ]

            # ---- filler machinery ----------------------------------------
            # All PSUM-chain generators (projections, output projections) are
            # emitted strictly FIFO through this deque so no two generators
            # ever interleave an open psum accumulation chain in the same
            # pool slot. ensure(label) pumps until the labeled generator has
            # fully emitted — used right before its first consumer's
            # emission, since emission order defines dependency direction.
            fillers = deque()  # labels
            gens = {}          # label -> generator
            done = set()

            def add_filler(label, gen):
                gens[label] = gen
                fillers.append(label)

            def pump(n):
                for _ in range(n):
                    while fillers:
                        try:
                            next(gens[fillers[0]])
                            break
                        except StopIteration:
                            done.add(fillers.popleft())
                    else:
                        return

            def ensure(label):
                if label not in gens or label in done:
                    return
                while label not in done:
                    assert fillers, f"ensure({label}): filler queue empty"
                    pump(1)

            def drain_now(gen):
                for _ in gen:
                    pass

            # ---- projection unit generators ------------------------------
            def proj_qk_units(w_tile, dst, ft, cq, which):
                ps = popool.tile([P, QC], F32, name=f"ps_{which}", tag="po")
                csl = slice(cq * QC, (cq + 1) * QC)
                for et in range(NE):
                    nc.tensor.matmul(
                        ps,
                        w_tile[:, et, ft * P : (ft + 1) * P],
                        xts[:, et, csl],
                        start=(et == 0),
                        stop=(et == NE - 1),
                    )
                    yield
                nc.vector.tensor_copy(dst[ft][:, csl], ps)
                yield

            def proj_v_units(st):
                vt = v_tiles[st]
                nc.vector.memset(vt[:, :, D : D + 1], 1.0)
                ps_v = popool.tile([P, FPC], F32, name="ps_v", tag="po")
                for et in range(NE):
                    nc.tensor.matmul(
                        ps_v,
                        xts[:, et, st * P : (st + 1) * P],
                        wv[:, et, :],
                        start=(et == 0),
                        stop=(et == NE - 1),
                    )
                    yield
                nc.vector.tensor_copy(
                    vt[:, :, 0:D], ps_v.rearrange("p (h d) -> p h d", d=D)
                )
                yield

            # ---- attention inner loop ------------------------------------
            def attn_core(pair, q0, W, per_kt=2, prev_fin=None, post_fin=None):
                """Heads 2*pair, 2*pair+1 for queries [q0, q0+W). Returns the
                two [P, W//P, P] psum accumulators (cols 0..63 = sum(exp*v),
                col 64 = sum(exp) per query-on-partition)."""
                nqt = W // P
                csl = slice(q0, q0 + W)
                oacc = [
                    oaccpool.tile([P, nqt, P], F32, name=f"oacc{s}", tag="oacc")
                    for s in range(2)
                ]

                def attnv(kt, et_t):
                    # One accumulation group per oacc tile (= one PSUM bank):
                    # start pending-zeroes the whole 2KB zero region, so only
                    # the first slice write may carry start; later qt slices'
                    # first writes land on pending-zero bytes (read-as-zero).
                    for sub in range(2):
                        for qt in range(nqt):
                            nc.tensor.matmul(
                                oacc[sub][:, qt, 0 : D + 1],
                                et_t[:, sub, qt * P : (qt + 1) * P],
                                v_tiles[kt][:, 2 * pair + sub, :],
                                start=(kt == 0 and qt == 0),
                                stop=(kt == NST - 1 and qt == nqt - 1),
                            )

                prev = None
                for kt in range(NST):
                    if kt % NQT == NQT - 1 and kt < NST - 1:
                        # k-projection for the NEXT chunk, prefetched one kt
                        # early so its copy lands before the scores need it.
                        ensure(f"k{pair}c{kt // NQT + 1}")
                    et_t = epool.tile([P, 2, W], F16, name="et_t", tag="et_t")
                    # ps_s keeps the full [P, 2, QC] shape so each head's
                    # slice stays within one psum bank for any W.
                    ps_s = pspool.tile([P, 2, QC], F32, name="ps_s", tag="ps_s")
                    for sub in range(2):
                        lo, hi = sub * D, (sub + 1) * D
                        nc.tensor.matmul(
                            ps_s[:, sub, 0:W],
                            kts[pair][lo:hi, kt * P : (kt + 1) * P],
                            qts[pair][lo:hi, csl],
                            start=True,
                            stop=True,
                        )
                    nc.scalar.activation(
                        out=et_t,
                        in_=ps_s[:, :, 0:W],
                        func=mybir.ActivationFunctionType.Exp,
                        scale=SM,
                    )
                    # v tile for kt must be emitted before attnv(kt), which
                    # is emitted next iteration.
                    ensure(f"v{kt}")
                    if kt == 1:
                        # previous phase's normalize/transpose, deferred past
                        # this phase's first (scores, exp) so it doesn't
                        # delay the exp stream at the boundary; must be fully
                        # emitted before attnv(kt0) below reuses oacc slots.
                        if prev_fin is not None:
                            drain_now(prev_fin)
                        if post_fin is not None:
                            add_filler(*post_fin)
                    # attnv for kt-1 is emitted after (scores, exp) of kt so
                    # the next exp's dependencies always outrank filler work.
                    if prev is not None:
                        attnv(*prev)
                        # fractional pumping: avg ~320ns (pair0) / ~213ns
                        # (pair1) of filler units per kt keeps PE/kt under
                        # the 1038ns exp cadence.
                        pump(per_kt if kt % 2 == 0 else max(1, per_kt - 1))
                    prev = (kt, et_t)
                attnv(*prev)
                return oacc

            def finish_units(pair, q0, W, oacc):
                """Normalize (fused psum evacuation) + transpose to [f, s]."""
                nqt = W // P
                rec = spool.tile([P, 2, nqt], F32, name="rec", tag="rec")
                o_sb = spool.tile([P, nqt, P], F16, name="o_sb", tag="o_sb")
                for sub in range(2):
                    nc.vector.reciprocal(rec[:, sub, :], oacc[sub][:, :, D : D + 1])
                    yield
                for qt in range(nqt):
                    for sub in range(2):
                        nc.vector.tensor_scalar_mul(
                            o_sb[:, qt, sub * D : (sub + 1) * D],
                            oacc[sub][:, qt, 0:D],
                            rec[:, sub, qt : qt + 1],
                        )
                        yield
                # pt lives in the oacc pool (not po): fin is emitted inline
                # between attn_cores and must not interleave with open po
                # chains from half-pumped filler generators.
                pt = oaccpool.tile([P, W], F16, name="pt", tag="oacc")
                for qt in range(nqt):
                    nc.tensor.transpose(
                        pt[:, qt * P : (qt + 1) * P], o_sb[:, qt, :], ident
                    )
                    yield
                nc.vector.tensor_copy(ots[pair][:, q0 : q0 + W], pt)
                yield

            def outproj_units(sts):
                """Output projection for s-tiles sts (needs both pairs' ots)."""
                for st in sts:
                    out_sb = outpool.tile([P, E], F16, name="out_sb", tag="out_sb")
                    for gc in range(2):
                        ps_out = popool.tile([P, QC], F32, name="ps_out", tag="po")
                        for ft in range(FT):
                            nc.tensor.matmul(
                                ps_out,
                                ots[ft][:, st * P : (st + 1) * P],
                                wo[:, ft, gc * QC : (gc + 1) * QC],
                                start=(ft == 0),
                                stop=(ft == FT - 1),
                            )
                            yield
                        nc.vector.tensor_copy(
                            out_sb[:, gc * QC : (gc + 1) * QC], ps_out
                        )
                        yield
                    nc.sync.dma_start(
                        out=out_d.ap()[st * P : (st + 1) * P, :], in_=out_sb
                    )

            def tail_finish_outproj(q0, W, oacc):
                """Last chunk: normalize/transpose/outproj pipelined per
                q-tile so the post-last-exp critical path stays short. ACT is
                idle here, so psum evacuations alternate ACT/DVE."""
                nqt = W // P
                rec = spool.tile([P, 2, nqt], F32, name="rec", tag="rec")
                for sub in range(2):
                    nc.vector.reciprocal(rec[:, sub, :], oacc[sub][:, :, D : D + 1])
                o_sb = spool.tile([P, nqt, P], F16, name="o_sb", tag="o_sb")
                for qt in range(nqt):
                    st = q0 // P + qt
                    # evacuate the two heads in parallel: sub0 on DVE, sub1
                    # on ACT (Copy with per-partition scale AP) — ACT is
                    # otherwise idle in the tail.
                    nc.vector.tensor_scalar_mul(
                        o_sb[:, qt, 0:D],
                        oacc[0][:, qt, 0:D],
                        rec[:, 0, qt : qt + 1],
                    )
                    nc.scalar.activation(
                        out=o_sb[:, qt, D : 2 * D],
                        in_=oacc[1][:, qt, 0:D],
                        func=mybir.ActivationFunctionType.Copy,
                        scale=rec[:, 1, qt : qt + 1],
                    )
                    # dedicated psum rings in the tail: transposes use po,
                    # out-projection psums use the (now idle) scores ring, so
                    # the two pipelines don't serialize on shared slots.
                    ptq = popool.tile([P, P], F16, name="ptq", tag="po")
                    nc.tensor.transpose(ptq, o_sb[:, qt, :], ident)
                    nc.scalar.activation(
                        out=ots[1][:, st * P : (st + 1) * P],
                        in_=ptq,
                        func=mybir.ActivationFunctionType.Copy,
                    )
                    out_sb = outpool.tile([P, E], F16, name="out_sb", tag="out_sb")
                    for gc in range(2):
                        ps_out = pspool.tile([P, QC], F32, name="ps_out", tag="ps_s")
                        for ft in range(FT):
                            nc.tensor.matmul(
                                ps_out,
                                ots[ft][:, st * P : (st + 1) * P],
                                wo[:, ft, gc * QC : (gc + 1) * QC],
                                start=(ft == 0),
                                stop=(ft == FT - 1),
                            )
                        if gc == 0:
                            nc.scalar.activation(
                                out=out_sb[:, gc * QC : (gc + 1) * QC],
                                in_=ps_out,
                                func=mybir.ActivationFunctionType.Copy,
                            )
                        else:
                            nc.vector.tensor_copy(
                                out_sb[:, gc * QC : (gc + 1) * QC], ps_out
                            )
                        nc.sync.dma_start(
                            out=out_d.ap()[
                                st * P : (st + 1) * P, gc * QC : (gc + 1) * QC
                            ],
                            in_=out_sb[:, gc * QC : (gc + 1) * QC],
                        )

            # ---- emission (= priority) -----------------------------------
            # startup: chunk-0 projections inline, rest as deadline-ordered
            # fillers (k before v before q; pair-1 weights later; outproj
            # appended as it becomes available).
            # chunk-0 k/q projections interleaved in et-halves: the first
            # half of each runs as soon as the first half of x lands.
            kg = proj_qk_units(wk, kts, 0, 0, "k0")
            qg = proj_qk_units(wq, qts, 0, 0, "q0")
            for _ in range(4):
                next(kg)
            for _ in range(4):
                next(qg)
            drain_now(kg)
            drain_now(qg)

            # deadline-ordered filler queue (k chunks gate the exp stream,
            # v tiles gate attnv, q chunks gate the next cq, pair-1 weights
            # gate the second half, outproj is deadline-free).
            for st in range(0, 4):
                add_filler(f"v{st}", proj_v_units(st))
            add_filler("k0c1", proj_qk_units(wk, kts, 0, 1, "k0"))
            add_filler("k0c2", proj_qk_units(wk, kts, 0, 2, "k0"))
            add_filler("k0c3", proj_qk_units(wk, kts, 0, 3, "k0"))
            for st in range(4, 8):
                add_filler(f"v{st}", proj_v_units(st))
            add_filler("q0c1", proj_qk_units(wq, qts, 0, 1, "q0"))
            for st in range(8, 16):
                add_filler(f"v{st}", proj_v_units(st))
            add_filler("q0c2", proj_qk_units(wq, qts, 0, 2, "q0"))
            add_filler("q0c3", proj_qk_units(wq, qts, 0, 3, "q0"))
            add_filler("k1c0", proj_qk_units(wk, kts, 1, 0, "k1"))
            add_filler("q1c0", proj_qk_units(wq, qts, 1, 0, "q1"))
            add_filler("k1c1", proj_qk_units(wk, kts, 1, 1, "k1"))
            add_filler("k1c2", proj_qk_units(wk, kts, 1, 2, "k1"))
            add_filler("k1c3", proj_qk_units(wk, kts, 1, 3, "k1"))
            # q1c1-3 sit behind the k1 chunks so they're left for pair 1's
            # loops, which otherwise run out of filler work.
            add_filler("q1c1", proj_qk_units(wq, qts, 1, 1, "q1"))
            add_filler("q1c2", proj_qk_units(wq, qts, 1, 2, "q1"))
            add_filler("q1c3", proj_qk_units(wq, qts, 1, 3, "q1"))

            # pair 1's last chunk is split 384+128 so the post-last-exp tail
            # (normalize/transpose/outproj) only covers one q-tile.
            ch0 = [(0, QC), (QC, QC), (2 * QC, QC), (3 * QC, QC)]
            phases = [(0, q0, W) for q0, W in ch0] + [(1, q0, W) for q0, W in ch0]
            pending_fin = None
            pending_outproj = None
            for i, (pair, q0, W) in enumerate(phases):
                last = i == len(phases) - 1
                ensure(f"q{pair}c{q0 // QC}")
                if pair == 1 and q0 == 0:
                    ensure("k1c0")
                oacc = attn_core(
                    pair,
                    q0,
                    W,
                    per_kt=2 if pair == 0 else 1,
                    prev_fin=pending_fin,
                    post_fin=pending_outproj,
                )
                pending_outproj = None
                if last:
                    while fillers:
                        pump(64)
                    tail_finish_outproj(q0, W, oacc)
                else:
                    pending_fin = finish_units(pair, q0, W, oacc)
                    if pair == 1:
                        sts = list(range(q0 // P, (q0 + W) // P))
                        pending_outproj = (f"outproj{q0}", outproj_units(sts))

    nc.compile()
    return nc


_NC_CACHE = None


def _get_nc():
    global _NC_CACHE
    if _NC_CACHE is None:
        _NC_CACHE = _build()
    return _NC_CACHE


def make_in_maps(x, Wq, Wk, Wv, Wo):
    in_maps = []
    xTs = [np.ascontiguousarray(x[b].T, dtype=np.float16) for b in range(B)]
    ident = np.eye(P, dtype=np.float16)
    for c in range(NCORES):
        b, hg = c // GPB, c % GPB
        fsl = slice(hg * FPC, (hg + 1) * FPC)
        in_maps.append({
            "xT": xTs[b],
            "wqT": np.ascontiguousarray(Wq[fsl, :].T, dtype=np.float16),
            "wkT": np.ascontiguousarray(Wk[fsl, :].T, dtype=np.float16),
            "wvT": np.ascontiguousarray(Wv[fsl, :].T, dtype=np.float16),
            "woT": np.ascontiguousarray(Wo[:, fsl].T, dtype=np.float16),
            "ident": ident,
        })
    return in_maps


def kernel(x, Wq, bq, Wk, bk, Wv, bv, Wo, bo):
    x = np.asarray(x, dtype=np.float32)
    Wq, Wk, Wv, Wo = (np.asarray(a, dtype=np.float32) for a in (Wq, Wk, Wv, Wo))
    bq, bk, bv, bo = (np.asarray(a, dtype=np.float32) for a in (bq, bk, bv, bo))
    if np.any(bq) or np.any(bk) or np.any(bv):
        # projection biases are zero for this problem spec; folding nonzero
        # biases into an augmented input row is not implemented.
        raise NotImplementedError("nonzero projection biases not supported")

    nc = _get_nc()
    in_maps = make_in_maps(x, Wq, Wk, Wv, Wo)
    res = run_bass_kernel_spmd(nc, in_maps, core_ids=list(range(NCORES)))
    out = np.empty((B, S, E), dtype=np.float32)
    for b in range(B):
        acc = res.results[b * GPB]["out"].astype(np.float32)
        for hg in range(1, GPB):
            acc += res.results[b * GPB + hg]["out"].astype(np.float32)
        out[b] = acc
    out += bo[None, None, :]
    return out
